# revision 6
# baseline (speedup 1.0000x reference)
"""Chamfer-distance (nn_CD_loss) Trainium2 kernel.

Computes reference:
    p1 = pixel2xyz(target), p2 = pixel2xyz(pred)   (N=16384 points each)
    D[i,j] = |p1_i|^2 + |p2_j|^2 - 2 p1_i.p2_j
    m12 = mean over valid i of min over valid j of D[i,j]
    m21 = mean over valid j of min over valid i of D[i,j]
    return m12 + m21

Strategy (8 NeuronCores, SPMD):
  Each core owns a 2048-row slice of each direction's distance matrix.
  The -2*p1.p2 inner products run on the PE at K=27 contraction built from an
  exact 3-way bf16 split of the fp32 coordinates (8 of 9 cross-product groups,
  dropping only lo*lo), plus 3 ones-rows carrying a 3-way bf16 split of the
  (validity-masked, +1e30) opposite-side squared norms.  PSUM tiles therefore
  hold E[i,j] = -2 p_i.q_j + sqq_masked[j] to ~1e-3 abs accuracy.
  The row-min runs on DVE via tensor_tensor_scan(min,min) consuming one PSUM
  tile + one SBUF tile per instruction (2 elem/lane/cycle); the SBUF side is
  staged by the Scalar engine copying alternating PSUM tiles.  The scan state
  chains across pair-iterations via initial=prev_out[:,-1:]; the last column
  of a row-block's final scan is its min.  Per-point mins return to the host,
  which adds the own-side squared norm and does the masked means (O(N) work).
"""

import numpy as np
import ml_dtypes

import concourse.bacc as bacc
import concourse.mybir as mybir
import concourse.tile as tile
from concourse.bass_utils import run_bass_kernel_spmd

H = W = 128
N = H * W                  # 16384 points per cloud
NCORES = 8
SHARE = N // NCORES        # 2048 rows per core per direction
BLOCKS = SHARE // 128      # 16 row-blocks of 128
K = 27                     # contraction: 8 product groups * 3 coords + 3 sq rows
CHUNK = 1024               # psum tile free size (2 banks)
PAIRS = N // (2 * CHUNK)   # TTR pair-iterations per row-block (8)
INF = np.float32(1.0e30)

_BF16 = ml_dtypes.bfloat16
# (lhs split level, rhs split level); 0=hi 1=mid 2=lo.  All 9 except (2,2).
_GROUPS = [(0, 0), (0, 1), (1, 0), (0, 2), (2, 0), (1, 1), (1, 2), (2, 1)]


def _pixel2xyz(depth, P):
    """depth [1,1,H,W] fp32 -> [N,3] fp32 (mirrors reference._pixel2xyz)."""
    d = depth[0, 0]
    px = np.broadcast_to(np.arange(W, dtype=np.float32)[None, :], (H, W))
    py = np.broadcast_to(np.arange(H, dtype=np.float32)[:, None], (H, W))
    c_u, c_v, f_u, f_v = P[0, 2], P[1, 2], P[0, 0], P[1, 1]
    x = (px * (d + P[2, 3]) - (c_u * d + P[0, 3])) / f_u
    y = (py * (d + P[2, 3]) - (c_v * d + P[1, 3])) / f_v
    return np.stack((x, y, d), axis=-1).reshape(-1, 3).astype(np.float32)


def _split3(v):
    """Exact 3-way bf16 split of fp32 array: v == h + m + l."""
    h = v.astype(_BF16)
    r = v - h.astype(np.float32)
    m = r.astype(_BF16)
    r2 = r - m.astype(np.float32)
    l = r2.astype(_BF16)
    return h, m, l


def _lhs_emb(Q):
    """Stationary-side embedding of point set Q [n,3] -> [K, n] bf16."""
    s = _split3(-2.0 * Q)          # each [n,3]
    rows = [s[a][:, c] for (a, _) in _GROUPS for c in range(3)]
    rows += [np.ones(Q.shape[0], dtype=_BF16)] * 3
    return np.stack(rows, axis=0)  # [27, n]


def _rhs_emb(R, sq_masked):
    """Moving-side embedding of point set R [n,3] + masked |R|^2 -> [K, n] bf16."""
    t = _split3(R)
    u = _split3(sq_masked)
    rows = [t[b][:, c] for (_, b) in _GROUPS for c in range(3)]
    rows += [u[0], u[1], u[2]]
    return np.stack(rows, axis=0)  # [27, n]


def build_program(chunk=CHUNK, psum_bufs=4, copy_bufs=3):
    """Build + compile the SPMD single-core program (same NEFF on all 8 cores)."""
    pairs = N // (2 * chunk)
    nc = bacc.Bacc("TRN2", target_bir_lowering=False, debug=False,
                   num_devices=NCORES)
    f32 = mybir.dt.float32
    bf16 = mybir.dt.bfloat16

    lhsA = nc.dram_tensor("lhsA", [K, SHARE], bf16, kind="ExternalInput")
    rhsA = nc.dram_tensor("rhsA", [K, N], bf16, kind="ExternalInput")
    lhsB = nc.dram_tensor("lhsB", [K, SHARE], bf16, kind="ExternalInput")
    rhsB = nc.dram_tensor("rhsB", [K, N], bf16, kind="ExternalInput")
    outA = nc.dram_tensor("outA", [128, BLOCKS], f32, kind="ExternalOutput")
    outB = nc.dram_tensor("outB", [128, BLOCKS], f32, kind="ExternalOutput")

    with tile.TileContext(nc) as tc:
        with (
            tc.tile_pool(name="const", bufs=1) as cpool,
            tc.tile_pool(name="psum", bufs=psum_bufs, space="PSUM") as ppool,
            tc.tile_pool(name="copies", bufs=copy_bufs) as copool,
            tc.tile_pool(name="scans", bufs=3) as apool,
            tc.tile_pool(name="gath", bufs=2) as gpool,
        ):
            lhsA_sb = cpool.tile([K, SHARE], bf16, tag="lhsA")
            rhsA_sb = cpool.tile([K, N], bf16, tag="rhsA")
            lhsB_sb = cpool.tile([K, SHARE], bf16, tag="lhsB")
            rhsB_sb = cpool.tile([K, N], bf16, tag="rhsB")
            minA = cpool.tile([128, BLOCKS], f32, tag="minA")
            minB = cpool.tile([128, BLOCKS], f32, tag="minB")
            nc.sync.dma_start(lhsA_sb[:], lhsA[:])
            for d0 in range(0, N, 4096):
                nc.sync.dma_start(rhsA_sb[:, d0:d0 + 4096],
                                  rhsA[:, d0:d0 + 4096])
            nc.sync.dma_start(lhsB_sb[:], lhsB[:])
            for d0 in range(0, N, 4096):
                nc.sync.dma_start(rhsB_sb[:, d0:d0 + 4096],
                                  rhsB[:, d0:d0 + 4096])

            for lhs_sb, rhs_sb, minbuf, out_dram in (
                (lhsA_sb, rhsA_sb, minA, outA),
                (lhsB_sb, rhsB_sb, minB, outB),
            ):
                for b in range(BLOCKS):
                    lhs_blk = lhs_sb[:, b * 128:(b + 1) * 128]
                    # Independent scans per pair-iteration (running min of the
                    # two tiles ends in the scratch's last column); the block
                    # min is a tiny reduce over the collected last columns.
                    acc = gpool.tile([128, pairs], f32, tag="acc")
                    for q in range(pairs):
                        base = q * 2 * chunk
                        pe_t = ppool.tile([128, chunk], f32, tag="ps")
                        for g in range(chunk // 512):
                            c0 = base + g * 512
                            nc.tensor.matmul(
                                pe_t[:, g * 512:(g + 1) * 512], lhs_blk,
                                rhs_sb[:, c0:c0 + 512], start=True, stop=True)
                        sb_t = copool.tile([128, chunk], f32, tag="cp")
                        nc.scalar.copy(sb_t[:], pe_t[:])
                        po_t = ppool.tile([128, chunk], f32, tag="ps")
                        for g in range(chunk // 512):
                            c0 = base + chunk + g * 512
                            nc.tensor.matmul(
                                po_t[:, g * 512:(g + 1) * 512], lhs_blk,
                                rhs_sb[:, c0:c0 + 512], start=True, stop=True)
                        sc = apool.tile([128, chunk], f32, tag="sc")
                        nc.vector.tensor_tensor_scan(
                            out=sc[:], data0=po_t[:], data1=sb_t[:],
                            initial=float(INF),
                            op0=mybir.AluOpType.min, op1=mybir.AluOpType.min)
                        nc.scalar.copy(acc[:, q:q + 1],
                                       sc[:, chunk - 1:chunk])
                    nc.vector.tensor_reduce(
                        minbuf[:, b:b + 1], acc[:], axis=mybir.AxisListType.X,
                        op=mybir.AluOpType.min)
                nc.sync.dma_start(out_dram[:], minbuf[:])
    nc.compile()
    return nc


def host_prep(pred, target, P_rect):
    pred = np.asarray(pred, dtype=np.float32)
    target = np.asarray(target, dtype=np.float32)
    P_rect = np.asarray(P_rect, dtype=np.float32)
    p1 = _pixel2xyz(target, P_rect)
    p2 = _pixel2xyz(pred, P_rect)
    valid = (target[0] > 0).reshape(-1)
    sq1 = np.sum(p1 * p1, axis=1).astype(np.float32)
    sq2 = np.sum(p2 * p2, axis=1).astype(np.float32)
    sq1m = np.where(valid, sq1, INF).astype(np.float32)
    sq2m = np.where(valid, sq2, INF).astype(np.float32)
    lhsA = np.ascontiguousarray(_lhs_emb(p1))      # rows = p1 points
    rhsA = np.ascontiguousarray(_rhs_emb(p2, sq2m))
    lhsB = np.ascontiguousarray(_lhs_emb(p2))      # rows = p2 points
    rhsB = np.ascontiguousarray(_rhs_emb(p1, sq1m))
    return p1, p2, valid, sq1, sq2, lhsA, rhsA, lhsB, rhsB


def finalize(results, valid, sq1, sq2):
    minA = np.concatenate(
        [np.asarray(results[c]["outA"]).T.reshape(-1) for c in range(NCORES)])
    minB = np.concatenate(
        [np.asarray(results[c]["outB"]).T.reshape(-1) for c in range(NCORES)])
    n = float(valid.sum())
    dist12 = sq1.astype(np.float64) + minA.astype(np.float64)
    dist21 = sq2.astype(np.float64) + minB.astype(np.float64)
    m12 = dist12[valid].sum() / n
    m21 = dist21[valid].sum() / n
    return np.asarray(np.float32(m12 + m21))


def kernel(pred, target, P_rect):
    p1, p2, valid, sq1, sq2, lhsA, rhsA, lhsB, rhsB = host_prep(
        pred, target, P_rect)
    nc = build_program()
    in_maps = []
    for c in range(NCORES):
        sl = slice(c * SHARE, (c + 1) * SHARE)
        in_maps.append({
            "lhsA": np.ascontiguousarray(lhsA[:, sl]),
            "rhsA": rhsA,
            "lhsB": np.ascontiguousarray(lhsB[:, sl]),
            "rhsB": rhsB,
        })
    res = run_bass_kernel_spmd(nc, in_maps, core_ids=list(range(NCORES)))
    return finalize(res.results, valid, sq1, sq2)
